# revision 9
# baseline (speedup 1.0000x reference)
"""Trainium2 Bass kernel for the BlackBoxSDE Euler-Maruyama scan.

Strategy: data-parallel over B across 8 cores (1024 trajectories/core).
Per core, the 1024 trajectories are blocked into 4 "lanes" of 256 that live
on PE-array row/col strips (partitions 32j+{0,1} hold state comps y0,y1 of
lane j).  Per time step:
  drift:  P_h = W1y^T @ yT   (K=2 row-tiled matmuls, f32r)
          e = Exp(P_h + c_n), r = Relu(P_h + c_n)      (ACT, per-step bias)
          m = min(e, 1)                                 (DVE)
          P_d (update delta) accumulates: dt*y1 rows, const row,
              dt*lam*W2^T r, dt*lam*alp*W2^T m          (PE accumulation)
  diff:   P_g = D1y^T @ yT; tanh; P_z = sqdt*D2^T tanh  (PE, lane-local)
  update: gz = P_z * z_n ; S' = (S + gz) + P_d          (DVE)
selu(x) = lam*relu(x) + lam*alp*(min(exp(x),1) - 1); the constant and bias
terms are folded into a per-step-independent const matmul row.
"""

import numpy as np

T, B, H, HD = 2048, 8192, 128, 32
NCORES = 8
BL = B // NCORES  # 1024 trajectories per core
LANES, LB = 4, 256  # lanes per core x trajectories per lane
SELU_L = 1.0507009873554804934193349852946
SELU_A = 1.6732632423543772848170429916717


def _interp_ut(t, u):
    """Match reference._interp_u(t, u, t[:-1]) exactly in fp32 numpy."""
    idx = np.clip(np.searchsorted(t, t[:-1]), 1, t.shape[0] - 1)
    t1, t2 = t[idx - 1], t[idx]
    u1, u2 = u[idx - 1], u[idx]
    alpha = ((t[:-1] - t1) / np.maximum(t2 - t1, np.float32(1e-6)))[:, None]
    return (u1 + alpha.astype(np.float32) * (u2 - u1)).astype(np.float32)


def host_tables(t, u, W1, b1, W2, b2, D1, d1, D2, d2):
    """Precompute all constant tables (fp32 numpy)."""
    f32 = np.float32
    dt = f32(t[1] - t[0])
    sqdt = f32(np.sqrt(dt))
    ut = _interp_ut(np.asarray(t, f32), np.asarray(u, f32))[:, 0]  # [T-1]
    Ts = T - 1
    lam, alp = f32(SELU_L), f32(SELU_A)

    # per-step ACT bias tables [128, T-1]
    c_all = (W1[2, :, None] * ut[None, :] + b1[:, None]).astype(f32)
    e_small = (D1[2, :, None] * ut[None, :] + d1[:, None]).astype(f32)  # [32,Ts]
    e4_all = np.zeros((128, Ts), f32)
    for j in range(LANES):
        e4_all[32 * j:32 * j + HD, :] = e_small

    # strip-replicated weights
    W1y_rep = np.zeros((128, H), f32)
    D1y_rep = np.zeros((128, HD), f32)
    D2s_rep = np.zeros((128, 2), f32)
    A0d_rep = np.zeros((128, 2), f32)
    Cb_rep = np.zeros((128, 2), f32)
    cb = dt * (f32(b2[0]) - lam * alp * f32(W2[:, 0].sum()))
    for j in range(LANES):
        W1y_rep[32 * j + 0, :] = W1[0, :]
        W1y_rep[32 * j + 1, :] = W1[1, :]
        D1y_rep[32 * j + 0, :] = D1[0, :]
        D1y_rep[32 * j + 1, :] = D1[1, :]
        D2s_rep[32 * j:32 * j + HD, :] = sqdt * D2
        A0d_rep[32 * j + 1, 0] = dt  # delta0 = dt * y1 ; delta1 linear part = 0
        Cb_rep[32 * j, 1] = cb       # const into delta1
    W2a = np.zeros((128, 2), f32)
    W2b = np.zeros((128, 2), f32)
    W2a[:, 1] = dt * lam * W2[:, 0]
    W2b[:, 1] = dt * lam * alp * W2[:, 0]
    return dict(c_all=c_all, e4_all=e4_all, W1y_rep=W1y_rep, D1y_rep=D1y_rep,
                D2s_rep=D2s_rep, A0d_rep=A0d_rep, Cb_rep=Cb_rep,
                W2a=W2a, W2b=W2b, dt=dt, sqdt=sqdt, ut=ut)


def shard_state(y0):
    """y0 [BL,2] -> S [128, 256]: S[32j+c, f] = y0[256j+f, c]."""
    S = np.zeros((128, LB), np.float32)
    for j in range(LANES):
        for c in range(2):
            S[32 * j + c, :] = y0[LB * j:LB * (j + 1), c]
    return S


def unshard_state(S):
    y = np.zeros((LB * LANES, 2), np.float32)
    for j in range(LANES):
        for c in range(2):
            y[LB * j:LB * (j + 1), c] = S[32 * j + c, :]
    return y


def _rnd_f32r(x, nbits=10):
    """Round-to-nearest keep-nbits-mantissa (tf32-ish model of f32r)."""
    xx = np.ascontiguousarray(np.asarray(x, np.float32))
    v = xx.view(np.uint32)
    shift = 23 - nbits
    mask = np.uint32(0xFFFFFFFF) << np.uint32(shift)
    return ((v + np.uint32(1 << (shift - 1))) & mask).view(np.float32)


def simulate_core(tbl, y0c, zc, n_steps, round_f32r=False,
                  round_state=None, round_mm=None, round_w=None, nbits=10):
    """Numpy simulator of the exact device algebra for one core.
    y0c [BL, 2], zc [n_steps, BL, 2].  Returns ys [n_steps+1, BL, 2].
    round_state: round stored state each step; round_mm: round matmul
    streaming inputs; round_w: round weight tables."""
    f32 = np.float32
    if round_state is None:
        round_state = round_f32r
    if round_mm is None:
        round_mm = round_f32r
    if round_w is None:
        round_w = round_f32r

    def rnd(x):
        return _rnd_f32r(x, nbits) if round_state else np.asarray(x, f32)

    def rmm(x):
        return _rnd_f32r(x, nbits) if round_mm else np.asarray(x, f32)

    if round_w:
        tbl = {k: (_rnd_f32r(v, nbits) if isinstance(v, np.ndarray) and
                   v.dtype == np.float32 and v.ndim >= 2 else v)
               for k, v in tbl.items()}

    S = shard_state(y0c)  # [128, 256]
    ys = np.zeros((n_steps + 1, BL, 2), f32)
    ys[0] = y0c
    lam, alp = f32(SELU_L), f32(SELU_A)
    for n in range(n_steps):
        P_h = np.zeros((128, BL), f32)
        P_g = np.zeros((128, LB), f32)
        P_d = np.zeros((128, LB), f32)
        P_z = np.zeros((128, LB), f32)
        Sr = rmm(S)
        for j in range(LANES):
            sj = Sr[32 * j:32 * j + 2, :]  # [2, 256]
            P_h[:, LB * j:LB * (j + 1)] = tbl["W1y_rep"][32 * j:32 * j + 2, :].T @ sj
            P_g[32 * j:32 * j + HD, :] = tbl["D1y_rep"][32 * j:32 * j + 2, :HD].T @ sj
            P_d[32 * j:32 * j + 2, :] = tbl["A0d_rep"][32 * j:32 * j + 2, :].T @ sj
            P_d[32 * j:32 * j + 2, :] += tbl["Cb_rep"][32 * j:32 * j + 1, :].T
        e = np.exp(P_h + tbl["c_all"][:, n:n + 1])
        r = np.maximum(P_h + tbl["c_all"][:, n:n + 1], 0.0)
        m = np.minimum(e, 1.0)
        th = np.tanh(P_g + tbl["e4_all"][:, n:n + 1])
        for j in range(LANES):
            sl = slice(LB * j, LB * (j + 1))
            P_d[32 * j:32 * j + 2, :] += tbl["W2a"].T @ rmm(r[:, sl])
            P_d[32 * j:32 * j + 2, :] += tbl["W2b"].T @ rmm(m[:, sl])
            P_z[32 * j:32 * j + 2, :] = (
                tbl["D2s_rep"][32 * j:32 * j + HD, :].T @ rmm(th[32 * j:32 * j + HD, :]))
        # z slab layout [128, 256] rows 32j+c = z[n, 256j+f, c]
        zt = shard_state(zc[n])
        gz = P_z * zt
        S = rnd((S + gz) + P_d)
        ys[n + 1] = unshard_state(S)
    return ys


# ---------------------------------------------------------------------------
# device kernel
# ---------------------------------------------------------------------------

SLAB = 32     # time steps staged per DMA slab
LBP = 264     # padded per-step slab pitch (so slab DMAs stay 3-D)
R_ACT = 512   # columns of the relu computed on ScalarE (rest on VectorE)


def build_program(n_steps):
    """Build the Bass/Tile program for one core, n_steps Euler-Maruyama steps."""
    from contextlib import ExitStack
    import concourse.bass as bass
    import concourse.tile as tile
    from concourse import bacc, mybir

    f32 = mybir.dt.float32
    f32r = mybir.dt.float32r
    AF = mybir.ActivationFunctionType

    nc = bacc.Bacc("TRN2", target_bir_lowering=False, debug=False)

    # DRAM I/O
    d_y0 = nc.dram_tensor("y0c", [BL, 2], f32, kind="ExternalInput")
    d_z = nc.dram_tensor("zc", [n_steps, 2, BL], f32, kind="ExternalInput")
    d_c = nc.dram_tensor("c_all", [128, n_steps], f32, kind="ExternalInput")
    d_e4 = nc.dram_tensor("e4_all", [128, n_steps], f32, kind="ExternalInput")
    bf16 = mybir.dt.bfloat16
    d_W1y = nc.dram_tensor("W1y", [128, H], bf16, kind="ExternalInput")
    d_D1y = nc.dram_tensor("D1y", [128, HD], bf16, kind="ExternalInput")
    d_D2s = nc.dram_tensor("D2s", [128, 2], bf16, kind="ExternalInput")
    d_A0d = nc.dram_tensor("A0d", [128, 2], bf16, kind="ExternalInput")
    d_Cb = nc.dram_tensor("Cb", [128, 2], bf16, kind="ExternalInput")
    d_Cg = nc.dram_tensor("Cg", [128, 2], bf16, kind="ExternalInput")
    d_W2a = nc.dram_tensor("W2a", [128, 2], bf16, kind="ExternalInput")
    d_W2b = nc.dram_tensor("W2b", [128, 2], bf16, kind="ExternalInput")
    d_ones = nc.dram_tensor("onesd", [128, LB], bf16, kind="ExternalInput")
    d_ys = nc.dram_tensor("ys", [n_steps + 1, 2, BL], f32, kind="ExternalOutput")

    with tile.TileContext(nc) as tc, ExitStack() as ctx:
        consts = ctx.enter_context(tc.tile_pool(name="consts", bufs=1))
        zpool = ctx.enter_context(tc.tile_pool(name="zslab", bufs=2))
        opool = ctx.enter_context(tc.tile_pool(name="oslab", bufs=2))
        work = ctx.enter_context(tc.tile_pool(name="work", bufs=2))
        lanew = ctx.enter_context(tc.tile_pool(name="lanew", bufs=2))
        pp = ctx.enter_context(tc.tile_pool(name="pp", bufs=1, space="PSUM"))

        # constant tables -> SBUF
        c_sb = consts.tile([128, n_steps], f32)
        nc.sync.dma_start(c_sb[:], d_c[:])
        e4_sb = consts.tile([128, n_steps], f32)
        nc.sync.dma_start(e4_sb[:], d_e4[:])

        def ld(dram, shape, tag):
            t = consts.tile(shape, bf16, tag=tag)
            nc.sync.dma_start(t[:], dram[:])
            return t

        W1y_s = ld(d_W1y, [128, H], "W1y_s")
        D1y_s = ld(d_D1y, [128, HD], "D1y_s")
        D2s_s = ld(d_D2s, [128, 2], "D2s_s")
        A0d_s = ld(d_A0d, [128, 2], "A0d_s")
        Cb_s = ld(d_Cb, [128, 2], "Cb_s")
        Cg_s = ld(d_Cg, [128, 2], "Cg_s")
        W2a_s = ld(d_W2a, [128, 2], "W2a_s")
        W2b_s = ld(d_W2b, [128, 2], "W2b_s")
        ones_s = ld(d_ones, [128, LB], "ones_s")

        # initial state: fp32 master + bf16 copy for matmul consumption
        S0m = consts.tile([128, LB], f32)
        for c in range(2):
            src = d_y0[:, c].rearrange("(j f) -> j f", f=LB)
            nc.sync.dma_start(S0m[c::32, :], src)
        S0b = consts.tile([128, LB], bf16)
        nc.vector.tensor_copy(S0b[:], S0m[:])

        # PSUM tiles (bufs=1; the state dependence serializes reuse)
        P_h = pp.tile([128, BL], f32)
        P_g = pp.tile([128, LB], f32)
        P_d = pp.tile([128, LB], f32)
        P_z = pp.tile([128, LB], f32)

        S_m, S_r = S0m, S0b
        n_slabs = (n_steps + SLAB - 1) // SLAB
        for s in range(n_slabs):
            n0 = s * SLAB
            ns = min(SLAB, n_steps - n0)
            z_sl = zpool.tile([128, SLAB, LBP], f32, tag="z")
            for c in range(2):
                nc.sync.dma_start(
                    z_sl[c::32, 0:ns, 0:LB],
                    d_z[n0:n0 + ns, c, :].rearrange("n (j f) -> j n f", f=LB),
                )
            o_sl = opool.tile([128, SLAB, LBP], f32, tag="o")
            for k in range(ns):
                n = n0 + k

                cb_n = c_sb[:, n:n + 1]
                # ---- PE: drift/diffusion preacts + linear delta ----
                for j in range(LANES):
                    r0, r1, r2 = 32 * j, 32 * j + 2, 32 * j + 32
                    nc.tensor.matmul(P_h[:, LB * j:LB * (j + 1)],
                                     W1y_s[r0:r1, :], S_r[r0:r1, :],
                                     start=True, stop=True,
                                     tile_position=(r0, 0))
                    nc.tensor.matmul(P_g[r0:r2, :], D1y_s[r0:r1, :],
                                     S_r[r0:r1, :], start=True, stop=True,
                                     tile_position=(r0, r0))
                    nc.tensor.matmul(P_d[r0:r1, :], A0d_s[r0:r1, :],
                                     S_r[r0:r1, :], start=True, stop=False,
                                     tile_position=(r0, r0))
                    nc.tensor.matmul(P_d[r0:r1, :], Cb_s[r0:r0 + 1, :],
                                     ones_s[r0:r0 + 1, :], start=False,
                                     stop=False, tile_position=(r0, r0))
                # ---- ACT: exp / relu(part) / tanh ----
                e_t = work.tile([128, BL], bf16, tag="e")
                nc.scalar.activation(e_t[:], P_h[:], AF.Exp, bias=cb_n)
                r_t = work.tile([128, BL], bf16, tag="r")
                if R_ACT > 0:
                    nc.scalar.activation(r_t[:, 0:R_ACT], P_h[:, 0:R_ACT],
                                         AF.Relu, bias=cb_n)
                if R_ACT < BL:
                    nc.vector.tensor_scalar(
                        out=r_t[:, R_ACT:BL], in0=P_h[:, R_ACT:BL],
                        scalar1=cb_n, scalar2=0.0,
                        op0=mybir.AluOpType.add, op1=mybir.AluOpType.max)
                m_t = work.tile([128, BL], bf16, tag="m")
                nc.vector.tensor_scalar_min(m_t[:], e_t[:], 1.0)
                th_t = lanew.tile([128, LB], bf16, tag="th")
                nc.scalar.activation(th_t[:], P_g[:], AF.Tanh,
                                     bias=e4_sb[:, n:n + 1])
                # ---- PE: thdd + diffusion output ----
                for j in range(LANES):
                    r0, r1, r2 = 32 * j, 32 * j + 2, 32 * j + 32
                    nc.tensor.matmul(P_d[r0:r1, :], W2a_s[:, :],
                                     r_t[:, LB * j:LB * (j + 1)],
                                     start=False, stop=False,
                                     tile_position=(0, r0))
                    nc.tensor.matmul(P_d[r0:r1, :], W2b_s[:, :],
                                     m_t[:, LB * j:LB * (j + 1)],
                                     start=False, stop=True,
                                     tile_position=(0, r0))
                    nc.tensor.matmul(P_z[r0:r1, :], D2s_s[r0:r2, :],
                                     th_t[r0:r2, :], start=True, stop=False,
                                     tile_position=(r0, r0))
                    nc.tensor.matmul(P_z[r0:r1, :], Cg_s[r0:r0 + 1, :],
                                     ones_s[r0:r0 + 1, :], start=False,
                                     stop=True, tile_position=(r0, r0))
                # ---- DVE: noise product + state update ----
                gz_t = lanew.tile([128, LB], f32, tag="gz")
                nc.vector.tensor_mul(gz_t[:], P_z[:], z_sl[:, k, 0:LB])
                t1_t = lanew.tile([128, LB], f32, tag="t1")
                nc.vector.tensor_add(t1_t[:], gz_t[:], S_m[:])
                Sr_t = lanew.tile([128, LB], bf16, tag="sr")
                nc.vector.tensor_add(Sr_t[:], t1_t[:], P_d[:])
                nc.vector.tensor_add(o_sl[:, k, 0:LB], t1_t[:], P_d[:])
                S_m = o_sl[:, k, 0:LB]
                S_r = Sr_t
            for c in range(2):
                nc.gpsimd.dma_start(
                    d_ys[1 + n0:1 + n0 + ns, c, :]
                    .rearrange("n (j f) -> j n f", f=LB),
                    o_sl[c::32, 0:ns, 0:LB],
                )
    nc.compile()
    return nc


def run_device(inputs, n_steps=None, trace=False):
    from concourse.bass_utils import run_bass_kernel_spmd

    f32 = np.float32
    if n_steps is None:
        n_steps = T - 1
    tbl = host_tables(
        inputs["t"], inputs["u"], inputs["W1"], inputs["b1"], inputs["W2"],
        inputs["b2"], inputs["D1"], inputs["d1"], inputs["D2"], inputs["d2"])
    sq = tbl["sqdt"]
    Cg_rep = np.zeros((128, 2), f32)
    for j in range(LANES):
        Cg_rep[32 * j, :] = sq * np.asarray(inputs["d2"], f32)
    import ml_dtypes
    bf = ml_dtypes.bfloat16
    common = dict(
        c_all=np.ascontiguousarray(tbl["c_all"][:, :n_steps]),
        e4_all=np.ascontiguousarray(tbl["e4_all"][:, :n_steps]),
        W1y=tbl["W1y_rep"].astype(bf), D1y=tbl["D1y_rep"].astype(bf),
        D2s=tbl["D2s_rep"].astype(bf), A0d=tbl["A0d_rep"].astype(bf),
        Cb=tbl["Cb_rep"].astype(bf), Cg=Cg_rep.astype(bf),
        W2a=tbl["W2a"].astype(bf), W2b=tbl["W2b"].astype(bf),
        onesd=np.ones((128, LB), bf),
    )
    y0 = np.asarray(inputs["y0"], f32)
    noise = np.asarray(inputs["noise"], f32)
    in_maps = []
    for k in range(NCORES):
        m = dict(common)
        m["y0c"] = np.ascontiguousarray(y0[k * BL:(k + 1) * BL])
        m["zc"] = np.ascontiguousarray(
            noise[:n_steps, k * BL:(k + 1) * BL].transpose(0, 2, 1))
        in_maps.append(m)

    nc = build_program(n_steps)
    res = run_bass_kernel_spmd(nc, in_maps, core_ids=list(range(NCORES)),
                               trace=trace)
    out = np.zeros((n_steps + 1, B, 2), f32)
    out[0] = y0
    for k in range(NCORES):
        ys = res.results[k]["ys"]  # [n_steps+1, 2, BL]
        for c in range(2):
            out[1:, k * BL:(k + 1) * BL, c] = ys[1:, c, :]
    return out, res


def kernel(**inputs):
    out, _ = run_device(inputs)
    return out


if __name__ == "__main__":
    pass
